# revision 60
# baseline (speedup 1.0000x reference)
"""Trainium2 Bass kernel for nn_Attention_11055245820093.

Swin-style attention block: qkv proj -> per-head scaled dot-product attention
with 2D relative position bias (CLS zero-padded), per-head softplus temperature,
patch-diagonal mask -> proj.

Strategy: data-parallel over batch B=64 across 8 NeuronCores (8 batches/core).
All compute per core runs in a "transposed" layout (channels on partitions,
tokens on the free dim) so no on-device transposes are needed:
  - Q^T/K^T projection computed in out^T (c, t) layout; scale/temp folded into
    the q weights; rel-pos bias/mask applied multiplicatively via a host-exp'd
    table
  - V computed in (t, c) layout directly (lhsT = x^T) over 13 dense
    cross-batch token chunks (full M=128 utilization), staged with a ones
    column after each head's 64 columns, then redistributed into per-batch
    tiles via partition-shifted SBUF->SBUF DMAs on the SP queue while the
    Q/K projections run; the AV matmul (M=65) then emits the softmax
    denominator as its 65th output row for free (no denominator matmuls)
  - S^T(j,i) = K^T.T @ Q^T per (batch,head); both j-chunks packed into one
    [128, 2N] PSUM tile so exp/mul run once per (batch, head)
  - softmax divide per (batch,head): DVE reciprocal of the denominator row,
    GpSimd partition-broadcast, then one DVE multiply that both divides and
    evacuates the AV PSUM into attn^T
  - proj consumes attn_out^T directly; v-bias and proj bias fold into a
    host-side constant added after gather.

Scheduling: phase C runs a 2-step-lookahead software pipeline per head-pair
(step b = [AV(b), QK(b+2), filler]); the fillers are the next head-pair's
Q/K projection groups (and, on the last pair, the first proj-output groups),
keeping the PE gap-free while exp/bias/divide latency hides. The hh=1 bias
multiply runs on GpSimd to keep DVE under the per-step budget. Input DMAs
alternate across the Pool/SP queues in consumption order. Matmuls in bf16
(fp32 PSUM accumulate); softmax math in fp32.
"""

import os
import sys

sys.path.insert(0, "/opt/trn_rl_repo")
os.environ.setdefault("MYCRO_LOCAL_CACHE", "1")

import numpy as np
import ml_dtypes

BF16 = ml_dtypes.bfloat16

# Problem constants (hardcoded per contract)
B, N, C, H, D = 64, 197, 768, 12, 64
NCORES = 8
BPC = B // NCORES          # 8 batches per core
T = BPC * N                # 1576 tokens per core
KT = C // 128              # 6 contraction tiles of 128
NT = 4                     # token n-tiles
TN = T // NT               # 394 tokens per n-tile
SCALE = D ** -0.5
N2 = 2 * N                 # 394

_CACHE = {}

TRACE = False
LAST_RESULTS = None
# fp8(e4m3) + DoubleRow for the QK^T matmuls (2x PE throughput there);
# Q and K each absorb sqrt(scale/temp) so their ranges match fp8 well
FP8QK = False


def _build(finalize=True):
    import concourse.bass as bass
    import concourse.tile as tile
    from concourse import bacc, mybir

    dt = mybir.dt
    f32, bf16 = dt.float32, dt.bfloat16
    AF = mybir.ActivationFunctionType

    nc = bacc.Bacc("TRN2", target_bir_lowering=False, debug=False)

    xT = nc.dram_tensor("xT", [KT, 128, T], bf16, kind="ExternalInput").ap()
    wqk = nc.dram_tensor("wqk", [KT, 128, 2 * C], bf16, kind="ExternalInput").ap()
    wv = nc.dram_tensor("wv", [KT, 128, C], bf16, kind="ExternalInput").ap()
    wpj = nc.dram_tensor("wpj", [KT, 128, C], bf16, kind="ExternalInput").ap()
    bT = nc.dram_tensor("bT", [KT, 128, 2 * N2], bf16, kind="ExternalInput").ap()
    bqk = nc.dram_tensor("bqk", [128, 2 * KT], f32, kind="ExternalInput").ap()
    outT = nc.dram_tensor("outT", [KT, 128, T], bf16, kind="ExternalOutput").ap()

    JROWS = (128, N - 128)  # 128, 69

    from concourse import library_config

    with tile.TileContext(nc) as tc:
        from contextlib import ExitStack

        nc.gpsimd.load_library(library_config.proxy)
        with ExitStack() as ctx:
            cp = ctx.enter_context(tc.tile_pool(name="consts", bufs=1))
            psA = ctx.enter_context(tc.tile_pool(name="psA", bufs=2, space="PSUM"))
            psQ = ctx.enter_context(tc.tile_pool(name="psQ", bufs=3, space="PSUM"))
            psP = ctx.enter_context(tc.tile_pool(name="psP", bufs=3, space="PSUM"))
            wp = ctx.enter_context(tc.tile_pool(name="work", bufs=2))

            # ---- persistent SBUF tiles ----
            x_sb = [cp.tile([128, T], bf16, name=f"x{k}", tag=f"x{k}") for k in range(KT)]
            wv_sb = [cp.tile([128, C], bf16, name=f"wv{k}", tag=f"wv{k}") for k in range(KT)]
            wqk_sb = [
                cp.tile([128, 2 * C], bf16, name=f"wqk{k}", tag=f"wqk{k}")
                for k in range(KT)
            ]
            wpj_sb = [
                cp.tile([128, C], bf16, name=f"wpj{k}", tag=f"wpj{k}")
                for k in range(KT)
            ]
            bqk_sb = cp.tile([128, 2 * KT], f32, name="bqk", tag="bqk")
            # bias per head-pair: [128, 2*2N]; hh block = [jt0 197 | jt1 197]
            bias_sb = [
                cp.tile([128, 2 * N2], bf16, name=f"bias{hp}", tag=f"bias{hp}")
                for hp in range(KT)
            ]
            # qk_sb[0:6] = Q^T tiles (c=0..767), qk_sb[6:12] = K^T tiles
            f8 = dt.float8e4
            qkdt = f8 if FP8QK else bf16
            qk_sb = [
                cp.tile([128, T], qkdt, name=f"qk{m}", tag=f"qk{m}")
                for m in range(2 * KT)
            ]
            # DoubleRow-packed Q/K: group g holds heads 4g..4g+3 (32
            # partitions each); free layout = [slot t: 2][T]: slot t carries
            # head-dim rows 32t..32t+31
            if FP8QK:
                # 3 heads per group tile (matmul operands may only start at
                # partition 0/32/64): head h -> tile h//3, partitions (h%3)*32
                qpk = [
                    cp.tile([96, 2 * T], f8, name=f"qpk{g}", tag=f"qpk{g}")
                    for g in range(H // 3)
                ]
                kpk = [
                    cp.tile([96, 2 * T], f8, name=f"kpk{g}", tag=f"kpk{g}")
                    for g in range(H // 3)
                ]
            # V per (batch, jt): [rows, 12*65]: head h at cols 65h..65h+64,
            # col 65h+64 = ones (gives softmax denominator via AV matmul)
            v_sb = {}
            for b in range(BPC):
                for jt, rows in enumerate(JROWS):
                    v_sb[(b, jt)] = cp.tile(
                        [rows, 65 * H], bf16, name=f"v{b}_{jt}", tag=f"v{b}_{jt}"
                    )
            attn_sb = [
                cp.tile([128, T], bf16, name=f"at{m}", tag=f"at{m}") for m in range(KT)
            ]
            # dense-V staging: 13 cross-batch token chunks, redistributed to
            # v_sb during the warmup/C(0) window (SP queue is idle there)
            NCH = (T + 127) // 128
            vstage = [
                cp.tile([128, 65 * H], bf16, name=f"vs{c}", tag=f"vs{c}")
                for c in range(NCH)
            ]

            # ---- input DMAs: consumption order, alternating between the Pool
            # and SP queues so descriptor generation issues twice as fast
            # (outT stores also use SP, but only much later) ----
            _dq = [nc.gpsimd, nc.sync]
            _dn = [0]

            def dma_in(out, in_):
                _dq[_dn[0] & 1].dma_start(out=out, in_=in_)
                _dn[0] += 1

            for k in range(KT):
                dma_in(x_sb[k][:, 0:TN], xT[k, :, 0:TN])
            for k in range(KT):
                dma_in(wv_sb[k][:, 0:384], wv[k, :, 0:384])
            for k in range(KT):
                dma_in(wv_sb[k][:, 384:C], wv[k, :, 384:C])
            for k in range(KT):
                dma_in(x_sb[k][:, TN : 2 * TN], xT[k, :, TN : 2 * TN])
            dma_in(bqk_sb[:], bqk[:])
            for k in range(KT):
                dma_in(wqk_sb[k][:, 0:C], wqk[k, :, 0:C])
            for k in range(KT):
                dma_in(x_sb[k][:, 2 * TN : 3 * TN], xT[k, :, 2 * TN : 3 * TN])
            for k in range(KT):
                dma_in(x_sb[k][:, 3 * TN : 4 * TN], xT[k, :, 3 * TN : 4 * TN])
            dma_in(bias_sb[0][:], bT[0])
            for k in range(KT):
                dma_in(wqk_sb[k][:, C : 2 * C], wqk[k, :, C : 2 * C])
            for hp in range(1, KT):
                nc.gpsimd.dma_start(out=bias_sb[hp][:], in_=bT[hp])
            for k in range(KT):
                nc.gpsimd.dma_start(out=wpj_sb[k][:], in_=wpj[k])

            # ---- helpers ----
            def emit_proj_group(mt, nt):
                ps = psA.tile([128, TN], f32, tag="mmA")
                for k in range(KT):
                    nc.tensor.matmul(
                        ps[:],
                        wqk_sb[k][:, mt * 128 : (mt + 1) * 128],
                        x_sb[k][:, nt * TN : (nt + 1) * TN],
                        start=(k == 0),
                        stop=(k == KT - 1),
                    )
                with nc.allow_low_precision(reason="fp8 Q/K for DoubleRow QK^T"):
                    nc.scalar.activation(
                        qk_sb[mt][:, nt * TN : (nt + 1) * TN],
                        ps[:],
                        AF.Identity,
                        bias=bqk_sb[:, mt : mt + 1],
                    )
                if FP8QK and nt in (1, 3):
                    # repack this half-row into DoubleRow layout: 4 partition-
                    # shifted SBUF->SBUF DMAs (heads hh, dim-halves s)
                    half = nt // 2
                    hp = mt % KT
                    pk = qpk if mt < KT else kpk
                    cl, cr = half * 2 * TN, (half + 1) * 2 * TN
                    for hh in range(2):
                        h = 2 * hp + hh
                        p0 = (h % 3) * 32
                        for s in range(2):
                            nc.sync.dma_start(
                                out=pk[h // 3][
                                    p0 : p0 + 32, s * T + cl : s * T + cr
                                ],
                                in_=qk_sb[mt][
                                    64 * hh + 32 * s : 64 * hh + 32 * s + 32, cl:cr
                                ],
                            )

            def c_qk(hp, b, hh):
                base = 64 * hh
                bN = b * N
                last = b == BPC - 1
                ps = psQ.tile([128, N2], f32, tag="qk")
                if FP8QK:
                    h = 2 * hp + hh
                    p0 = (h % 3) * 32
                    kv = kpk[h // 3][p0 : p0 + 32, :].rearrange(
                        "p (t j) -> p t j", t=2
                    )
                    qv = qpk[h // 3][p0 : p0 + 32, :].rearrange(
                        "p (t j) -> p t j", t=2
                    )
                    DR = mybir.MatmulPerfMode.DoubleRow
                    mm0 = lambda o, l: nc.tensor.matmul(
                        o, l, qv[:, :, bN : bN + N], start=True, stop=True,
                        perf_mode=DR,
                    )
                    mm0(ps[0:128, 0:N], kv[:, :, bN : bN + 128])
                    if not last:
                        # M=128: rows 69.. are next-batch keys (finite junk;
                        # their e-rows get zeroed by the host bias, unread)
                        mm0(ps[0:128, N:N2], kv[:, :, bN + 128 : bN + 256])
                    else:
                        mm0(ps[0:69, N:N2], kv[:, :, bN + 128 : bN + N])
                else:
                    nc.tensor.matmul(
                        ps[0:128, 0:N],
                        qk_sb[KT + hp][base : base + 64, bN : bN + 128],
                        qk_sb[hp][base : base + 64, bN : bN + N],
                        start=True,
                        stop=True,
                    )
                    nc.tensor.matmul(
                        ps[0 : (69 if last else 128), N:N2],
                        qk_sb[KT + hp][
                            base : base + 64,
                            bN + 128 : bN + (N if last else 256),
                        ],
                        qk_sb[hp][base : base + 64, bN : bN + N],
                        start=True,
                        stop=True,
                    )
                eu = wp.tile([128, N2], bf16, tag="eu", bufs=3)
                e = wp.tile([128, N2], bf16, tag="e", bufs=5)
                # hh=1 bias-mul on GpSimd (SBUF-only) to unload DVE
                eng = nc.vector if hh == 0 else nc.gpsimd
                if not last:
                    nc.scalar.activation(eu[:], ps[0:128, 0:N2], AF.Exp)
                    eng.tensor_mul(
                        e[:], eu[:], bias_sb[hp][:, hh * N2 : (hh + 1) * N2]
                    )
                else:
                    # split exp/mul: avoid reading the unwritten PSUM rows
                    nc.scalar.activation(eu[0:128, 0:N], ps[0:128, 0:N], AF.Exp)
                    nc.scalar.activation(eu[0:69, N:N2], ps[0:69, N:N2], AF.Exp)
                    eng.tensor_mul(
                        e[0:128, 0:N],
                        eu[0:128, 0:N],
                        bias_sb[hp][0:128, hh * N2 : hh * N2 + N],
                    )
                    eng.tensor_mul(
                        e[0:69, N:N2],
                        eu[0:69, N:N2],
                        bias_sb[hp][0:69, hh * N2 + N : hh * N2 + N2],
                    )
                return e

            def c_av(hp, b, hh, e):
                h = 2 * hp + hh
                bN = b * N
                po = psP.tile([65, N], f32, tag="po")
                nc.tensor.matmul(
                    po[0:65, 0:N],
                    v_sb[(b, 0)][0:128, 65 * h : 65 * h + 65],
                    e[0:128, 0:N],
                    start=True,
                    stop=False,
                )
                nc.tensor.matmul(
                    po[0:65, 0:N],
                    v_sb[(b, 1)][0:69, 65 * h : 65 * h + 65],
                    e[0:69, N:N2],
                    start=False,
                    stop=True,
                )
                # per-(b,h) softmax division: reciprocal of the ones-column
                # row, broadcast to 64 partitions (GpSimd), then a single DVE
                # mul that both divides and evacuates PSUM -> attn_sb
                rc = wp.tile([1, N], bf16, tag="rc", bufs=4)
                with nc.allow_low_precision(
                    reason="softmax denom reciprocal in bf16"
                ):
                    nc.vector.reciprocal(rc[0:1, 0:N], po[64:65, 0:N])
                rbs = wp.tile([64, N], bf16, tag="rbs", bufs=4)
                nc.gpsimd.partition_broadcast(rbs[0:64, 0:N], rc[0:1, 0:N])
                nc.vector.tensor_mul(
                    attn_sb[hp][64 * hh : 64 * hh + 64, bN : bN + N],
                    po[0:64, 0:N],
                    rbs[0:64, 0:N],
                )

            # ---- Phase B: V via dense cross-batch token chunks into the
            # persistent stage tiles (13x128 instead of 16 per-batch chunks) ----
            for n2 in range(2):
                for c in range(NCH):
                    t0, t1 = 128 * c, min(128 * c + 128, T)
                    rows = t1 - t0
                    stage = vstage[c]
                    psv = psA.tile([128, 384], f32, tag="mmA")
                    for k in range(KT):
                        nc.tensor.matmul(
                            psv[0:rows, 0:384],
                            x_sb[k][:, t0:t1],
                            wv_sb[k][:, n2 * 384 : (n2 + 1) * 384],
                            start=(k == 0),
                            stop=(k == KT - 1),
                        )
                    dst = stage[0:rows, n2 * 390 : (n2 + 1) * 390].rearrange(
                        "p (h x) -> p h x", h=6
                    )[:, :, 0:64]
                    src = psv[0:rows, 0:384].rearrange("p (h x) -> p h x", h=6)
                    nc.scalar.activation(dst, src, AF.Copy)
                    if n2 == 1:
                        ones_view = stage[0:rows, :].rearrange(
                            "p (h x) -> p h x", h=H
                        )[:, :, 64:65]
                        nc.vector.memset(ones_view, 1.0)

            # redistribute stage rows into per-(b,jt) V tiles: partition-
            # shifted SBUF->SBUF DMAs on the SP queue, overlapping warmup/C(0)
            vbounds = sorted(
                {b * N for b in range(BPC)} | {b * N + 128 for b in range(BPC)} | {T}
            )
            for c in range(NCH):
                t0, t1 = 128 * c, min(128 * c + 128, T)
                a = t0
                for bd in vbounds:
                    if bd <= t0:
                        continue
                    e_ = min(bd, t1)
                    b, j = a // N, a % N
                    jt = 1 if j >= 128 else 0
                    nc.sync.dma_start(
                        out=v_sb[(b, jt)][
                            j - 128 * jt : j - 128 * jt + (e_ - a), :
                        ],
                        in_=vstage[c][a - t0 : e_ - t0, :],
                    )
                    a = e_
                    if a >= t1:
                        break

            # ---- Phase A warmup: Q^T/K^T for head-pair 0 ----
            for nt in range(NT):
                emit_proj_group(0, nt)
                emit_proj_group(KT + 0, nt)

            # ---- Phase D group (proj); used standalone and woven into C ----
            def emit_d_group(mt, nt, halves=1):
                ps = psA.tile([128, TN], f32, tag="mmA")
                for k in range(KT):
                    nc.tensor.matmul(
                        ps[:],
                        wpj_sb[k][:, mt * 128 : (mt + 1) * 128],
                        attn_sb[k][:, nt * TN : (nt + 1) * TN],
                        start=(k == 0),
                        stop=(k == KT - 1),
                    )
                ot = wp.tile([128, TN], bf16, tag="ot", bufs=3)
                hw = TN // halves
                for i in range(halves):
                    nc.scalar.activation(
                        ot[:, i * hw : (i + 1) * hw], ps[:, i * hw : (i + 1) * hw],
                        AF.Copy,
                    )
                    # pin the very last store to the quiet Pool queue so its
                    # descriptor generation idles at the semaphore before the
                    # copy finishes (shortest possible tail)
                    q = nc.gpsimd if (halves > 1 and i == halves - 1) else None
                    if q is None:
                        dma_in(
                            outT[mt, :, nt * TN + i * hw : nt * TN + (i + 1) * hw],
                            ot[:, i * hw : (i + 1) * hw],
                        )
                    else:
                        q.dma_start(
                            out=outT[
                                mt, :, nt * TN + i * hw : nt * TN + (i + 1) * hw
                            ],
                            in_=ot[:, i * hw : (i + 1) * hw],
                        )

            # ---- Phase C per head-pair, weaving in A for the next pair ----
            # 2-step QK lookahead: step b runs [AV(b), QK(b+2), filler group]
            d_left = [(mt, nt) for nt in range(NT) for mt in range(KT)]
            for hp in range(KT):
                if hp + 1 < KT:
                    fillers = [(hp + 1, nt) for nt in range(NT)] + [
                        (KT + hp + 1, nt) for nt in range(NT)
                    ]
                    fill = lambda b: emit_proj_group(*fillers.pop(0)) if fillers else None
                else:
                    # last head-pair: weave the six nt0 proj-output groups
                    # (nt0 inputs are fully divided once b0/b1 are done)
                    def fill(b):
                        if b >= 2 and d_left and d_left[0][1] == 0:
                            emit_d_group(*d_left.pop(0))
                e_pend = {}
                for b in range(2):
                    for hh in range(2):
                        e_pend[(b, hh)] = c_qk(hp, b, hh)
                for b in range(BPC):
                    for hh in range(2):
                        c_av(hp, b, hh, e_pend.pop((b, hh)))
                    if b + 2 < BPC:
                        for hh in range(2):
                            e_pend[(b + 2, hh)] = c_qk(hp, b + 2, hh)
                    fill(b)

            # ---- Phase D: remaining proj groups (final one split for a
            # shorter post-matmul store tail) ----
            while d_left:
                mt, nt = d_left.pop(0)
                emit_d_group(mt, nt, halves=2 if not d_left else 1)

    if finalize:
        nc.finalize()
    return nc


def _host_prep(x, qkv_w, qkv_b, proj_w, proj_b, rel_table, log_temp, rel_index):
    """Build the per-core input maps (host-side layout prep only)."""
    x = np.asarray(x, np.float32)
    qkv_w = np.asarray(qkv_w, np.float32)
    qkv_b = np.asarray(qkv_b, np.float32)
    proj_w = np.asarray(proj_w, np.float32)
    rel_table = np.asarray(rel_table, np.float32)
    log_temp = np.asarray(log_temp, np.float32)
    rel_index = np.asarray(rel_index)

    temp = np.log1p(np.exp(log_temp.astype(np.float64))).astype(np.float32)  # softplus
    alpha = (SCALE / temp).astype(np.float32)         # (H,) folded into q (+k)
    alpha_c = np.repeat(alpha, D)                     # (768,)

    wqkT = qkv_w[0 : 2 * C].T.copy()                  # (768, 1536)
    if FP8QK:
        # split the scale evenly so Q and K have matched fp8 ranges
        sa_c = np.sqrt(alpha_c)
        wqkT[:, 0:C] *= sa_c[None, :]
        wqkT[:, C : 2 * C] *= sa_c[None, :]
        bq = qkv_b[0:C] * sa_c
        bk = qkv_b[C : 2 * C] * sa_c
    else:
        wqkT[:, 0:C] *= alpha_c[None, :]
        bq = qkv_b[0:C] * alpha_c
        bk = qkv_b[C : 2 * C]
    wqk_np = wqkT.reshape(KT, 128, 2 * C).astype(BF16)

    wv_np = qkv_w[2 * C : 3 * C].T.reshape(KT, 128, C).astype(BF16)
    wpj_np = proj_w.T.reshape(KT, 128, C).astype(BF16)

    bqk_np = np.concatenate([bq, bk]).reshape(2 * KT, 128).T.copy().astype(np.float32)

    # multiplicative bias table: exp((relpos bias)/temp), diag -> 0, CLS -> 1,
    # transposed to (j, i); packed per head-pair as
    # [128, (hh0: jt0 | jt1), (hh1: jt0 | jt1)] with jt1 rows 69.. zeroed
    rpb = rel_table[rel_index]                        # (196, 196, H)
    bias = np.zeros((H, N, N), np.float32)
    bias[:, 1:, 1:] = rpb.transpose(2, 0, 1) / temp[:, None, None]
    ebias = np.exp(bias)
    idx = np.arange(1, N)
    ebias[:, idx, idx] = 0.0
    ebT = ebias.transpose(0, 2, 1)                    # (H, j, i)
    bT_np = np.zeros((KT, 128, 2 * N2), np.float32)
    for hp in range(KT):
        for hh in range(2):
            h = 2 * hp + hh
            bT_np[hp, 0:128, hh * N2 : hh * N2 + N] = ebT[h, 0:128, :]
            bT_np[hp, 0:69, hh * N2 + N : hh * N2 + N2] = ebT[h, 128:N, :]
    bT_np = bT_np.astype(BF16)

    in_maps = []
    for c in range(NCORES):
        xc = x[c * BPC : (c + 1) * BPC].reshape(T, C).T  # (768, T)
        xT_np = xc.reshape(KT, 128, T).astype(BF16)
        in_maps.append(
            {
                "xT": xT_np,
                "wqk": wqk_np,
                "wv": wv_np,
                "wpj": wpj_np,
                "bT": bT_np,
                "bqk": bqk_np,
            }
        )
    return in_maps


def kernel(**inputs) -> np.ndarray:
    global LAST_RESULTS
    from concourse.bass_utils import run_bass_kernel_spmd

    if "nc" not in _CACHE:
        _CACHE["nc"] = _build()
    nc = _CACHE["nc"]

    in_maps = _host_prep(**inputs)
    try:
        res = run_bass_kernel_spmd(
            nc, in_maps, core_ids=list(range(NCORES)), trace=TRACE
        )
    except ModuleNotFoundError:
        res = run_bass_kernel_spmd(
            nc, in_maps, core_ids=list(range(NCORES)), trace=False
        )
    LAST_RESULTS = res

    # v-bias rides through attention unchanged (rows of attn sum to 1), so
    # its proj image folds into the constant output bias added here
    proj_b = np.asarray(inputs["proj_b"], np.float32)
    proj_w = np.asarray(inputs["proj_w"], np.float32)
    bv = np.asarray(inputs["qkv_b"], np.float32)[2 * C : 3 * C]
    b_eff = proj_b + proj_w @ bv
    outs = []
    for c in range(NCORES):
        oT = np.asarray(res.results[c]["outT"], np.float32).reshape(C, T)
        outs.append(oT.T.reshape(BPC, N, C))
    out = np.concatenate(outs, axis=0) + b_eff[None, None, :]
    return out.astype(np.float32)
